# revision 24
# baseline (speedup 1.0000x reference)
"""Trainium2 Bass kernel for nn_CosSim (segment-mean + own-class cosine sim).

cos_i = <f_i, m_{l_i}> / (||f_i|| * ||m_{l_i}||),  m_c = mean of rows with label c.

cos is invariant to positive scaling of m, so m_hat_c = gsums_c/||gsums_c||
(per-class counts cancel; no count handling on device at all).

Hybrid data-parallel strategy (8 cores, AllReduce for class sums). Tiles
[0, XC) are "C-tiles" (use host-shipped F^T), tiles [XC, T) are "A-tiles"
(row-major F only):
  host:  fp16 feat shard fb [128, T*D] (tile-major), F^T ft [128, 4*RPAD],
         one-hot h [128, T*C], transposed one-hot ht [100, XA*128] for
         A-tiles, identity id_h [128,128], all fp16.
  p1:    per tile: sums[100,512] += H_t^T @ F_t (fp16 matmul, PSUM f32).
         A-tile norms: ACT Square+accum -> nf2. C-tile norms: Gram
         FFT_t = sum_s F_s F_s^T on PE (from ft), diagonal extracted by a
         narrow DVE tensor_tensor_reduce against the identity -> nf2
         (runs during the AllReduce window).
  AR:    AllReduce the [100,512] sums in fp16.
  p2:    m_hat = gsums * rsqrt(rowsum(gsums^2)) in fp16; PE-transpose to
         mnt [128, 4*100] for the C-tile S matmuls.
  p3:    C-tile: S_t[128,100] = sum_s F_s^T^T @ mnt_s (4 fp16 matmuls);
         dot = TTR(h_t, S_t) -> dots[:,t]   (229 ns)
         A-tile: Cent_t[128,512] = Ht_t^T @ m_hat (1 fp16 matmul);
         dot = TTR(F_t, Cent_t) -> dots[:,t] (615 ns)
         cos = dots * rsqrt(nf2).
  reps>1: rep r+1's p1 (fb DMA + matmuls + Gram) is interleaved into rep
  r's p3 so PE stays ramped and the collective overlaps compute.
"""
import sys
import os

sys.path.insert(0, "/opt/trn_rl_repo")

import numpy as np

import concourse.bacc as bacc
import concourse.tile as tile
import concourse.mybir as mybir
import concourse.bass_utils as bass_utils

F32 = mybir.dt.float32
FP16 = mybir.dt.float16
AL = mybir.AluOpType
AF = mybir.ActivationFunctionType
AX = mybir.AxisListType

N = 100000
D = 512
C = 100
NCORES = 8
RPC = N // NCORES            # 12500 real rows per core
T = (RPC + 127) // 128       # 98 tiles per core
RPAD = T * 128               # 12544 padded rows
CH = 14                      # tiles per fb DMA chunk
NCH = T // CH                # 7 chunks
assert NCH * CH == T

XC = int(os.environ.get("KERNEL_XC", "98"))        # C-tiles (use F^T)
XA = T - XC
# F^T DMA chunking: FCH tiles per chunk
FCH = next((f for f in (13, 14, 12, 16, 10, 8, 7, 4, 2, 1) if XC % f == 0),
           max(XC, 1))
NFC = XC // FCH if XC else 0
ILV = os.environ.get("KERNEL_ILV", "1") == "1"     # cross-rep PE interleave
ILVK = int(os.environ.get("KERNEL_ILVK", "1"))     # interleave steps per tile
ILAG = int(os.environ.get("KERNEL_ILAG", str(CH)))  # slots before interleave
ARDT = os.environ.get("KERNEL_ARDT", "f32")          # AllReduce dtype
# TensorTensorReduce crashes the real NEFF path (works in CoreSim) -- never
# emit it. Mults go to DVE (or Pool with KERNEL_POOL=1), reductions split
# between DVE tensor_reduce and ACT Identity-accumulate.
POOLTT = os.environ.get("KERNEL_POOL", "0") == "1"   # gram/C mults on Pool
AMOD = int(os.environ.get("KERNEL_AMOD", "99"))      # A-red: t%AMOD==0 on DVE
CRED = os.environ.get("KERNEL_CRED", "act")          # C-dot reduction engine
GMOD = int(os.environ.get("KERNEL_GMOD", "2"))       # G-red: t%GMOD==0 on DVE

LAST_RESULTS = None  # BassKernelResults of the last run (for test.py)


def _build_program(reps: int = 1):
    """reps>1 repeats the whole pipeline for steady-state delta timing."""
    nc = bacc.Bacc("TRN2", target_bir_lowering=False, debug=False,
                   num_devices=NCORES)

    fb_d = nc.dram_tensor("fb", [128, T * D], FP16, kind="ExternalInput").ap()
    h_d = nc.dram_tensor("h", [128, T * C], FP16, kind="ExternalInput").ap()
    id_d = nc.dram_tensor("idh", [128, 128], FP16, kind="ExternalInput").ap()
    if_d = nc.dram_tensor("idf", [128, 128], F32, kind="ExternalInput").ap()
    ARD = F32 if ARDT == "f32" else FP16
    if XC:
        ft_d = nc.dram_tensor("ft", [128, 4 * RPAD], FP16,
                              kind="ExternalInput").ap()
        ft_rv = ft_d.rearrange("p (s r) -> p s r", s=4)   # [128, 4, RPAD]
    if XA:
        ht_d = nc.dram_tensor("ht", [C, XA * 128], FP16,
                              kind="ExternalInput").ap()
    out_d = nc.dram_tensor("out", [128, T], F32, kind="ExternalOutput").ap()

    fb_rv = fb_d.rearrange("p (t d) -> p t d", d=D)   # [128, T, D]

    with tile.TileContext(nc) as tc:
        with (
            tc.tile_pool(name="fbp", bufs=(3 if XA == 0 else NCH)) as fb_pool,
            tc.tile_pool(name="ftp", bufs=max(NFC, 1)) as ft_pool,
            tc.tile_pool(name="res", bufs=1) as res_pool,
            tc.tile_pool(name="sq", bufs=3) as sq_pool,
            tc.tile_pool(name="tto", bufs=3) as tto_pool,
            tc.tile_pool(name="nto", bufs=3) as nto_pool,
            tc.tile_pool(name="ps_sums", bufs=2, space="PSUM") as ps_sums_pool,
            tc.tile_pool(name="ps_wrk", bufs=int(os.environ.get("KERNEL_PSW", "4")), space="PSUM") as ps_wrk_pool,
            tc.tile_pool(name="ps_tp", bufs=2, space="PSUM") as ps_tp_pool,
            tc.tile_pool(name="dram", bufs=2, space="DRAM") as dram_pool,
        ):
            # ---- resident tensors ----
            h_res = res_pool.tile([128, T * C], FP16, tag="h_res")
            h_v = h_res[:].rearrange("p (t c) -> p t c", c=C)
            id_h = res_pool.tile([128, 128], FP16, tag="id_h")
            if XA:
                ht_res = res_pool.tile([C, XA * 128], FP16, tag="ht_res")
                ht_v = ht_res[:].rearrange("c (t p) -> c t p", p=128)
            nf2 = res_pool.tile([128, T], F32, tag="nf2")
            rnf = res_pool.tile([128, T], F32, tag="rnf")
            dots = res_pool.tile([128, T], F32, tag="dots")
            cos = res_pool.tile([128, T], F32, tag="cos")
            sums_h = res_pool.tile([C, D], ARD, tag="sums_h")
            gsum_h = res_pool.tile([C, D], ARD, tag="gsum_h")
            gscr = res_pool.tile([C, D], FP16, tag="gscr")
            gmn_f = res_pool.tile([C, D], F32, tag="gmn_f")
            gmn_h = res_pool.tile([C, D], FP16, tag="gmn_h")
            id_f = res_pool.tile([128, 128], F32, tag="id_f")
            nm2 = res_pool.tile([C, 1], F32, tag="nm2")
            rnm = res_pool.tile([C, 1], F32, tag="rnm")
            mnt = res_pool.tile([128, 4 * C], FP16, tag="mnt")
            mnt_v = mnt[:].rearrange("p (s c) -> p s c", c=C)

            nc.sync.dma_start(h_res[:], h_d[:])
            nc.sync.dma_start(id_h[:], id_d[:])
            nc.sync.dma_start(id_f[:], if_d[:])
            if XA:
                nc.sync.dma_start(ht_res[:], ht_d[:])

            # ---- phase-1 work emitters (rep 0 standalone, then interleaved)
            fb_bufs = [None] * NCH       # rotating fb chunk views
            ft_bufs = [None] * max(NFC, 1)   # rotating F^T chunk views
            sums_of_rep = {}

            def p1_dma_chunk(k):
                fb_c = fb_pool.tile([128, CH * D], FP16, tag="fb")
                fb_cv = fb_c[:].rearrange("p (j d) -> p j d", d=D)
                nc.sync.dma_start(fb_cv[:, :, :],
                                  fb_rv[:, k * CH:(k + 1) * CH, :])
                fb_bufs[k] = fb_cv

            def p1_mm(rep, t):
                nc.tensor.matmul(
                    sums_of_rep[rep][:],
                    lhsT=h_v[:, t, :],
                    rhs=fb_bufs[t // CH][:, t % CH, :],
                    start=(t == 0),
                    stop=(t == T - 1),
                )

            def a_norm(t):
                # ||f_t||^2 on ACT: Square with row-accumulate
                sq = sq_pool.tile([128, D], FP16, tag="sq")
                nc.scalar.activation(sq[:], fb_bufs[t // CH][:, t % CH, :],
                                     AF.Square, accum_out=nf2[:, t:t + 1])

            def ft_dma_chunk(fc):
                ft_c = ft_pool.tile([128, 4 * FCH * 128], FP16, tag="ftc")
                ft_cv = ft_c[:].rearrange("p (s r) -> p s r", s=4)
                (nc.gpsimd if os.environ.get("KERNEL_FTQ", "sync") == "pool"
                 else nc.sync).dma_start(
                    ft_cv[:, :, :],
                    ft_rv[:, :, fc * FCH * 128:(fc + 1) * FCH * 128])
                ft_bufs[fc] = ft_cv

            def c_gram(t):
                # FFT_t = sum_s F_s F_s^T; diag via TTR against identity
                fc, j = t // FCH, t % FCH
                wrk = ps_wrk_pool.tile([128, D], F32, name="wrk", tag="wrk")
                fft = wrk[:, :128]
                for s in range(4):
                    sl = ft_bufs[fc][:, s, j * 128:(j + 1) * 128]
                    nc.tensor.matmul(fft, lhsT=sl, rhs=sl,
                                     start=(s == 0), stop=(s == 3))
                nto = nto_pool.tile([128, 128], FP16, tag="nto")
                eng = nc.gpsimd if POOLTT else nc.vector
                eng.tensor_tensor(out=nto[:], in0=id_h[:], in1=fft,
                                  op=AL.mult)
                if t % GMOD == 0:
                    nc.vector.tensor_reduce(out=nf2[:, t:t + 1], in_=nto[:],
                                            axis=AX.X, op=AL.add)
                else:
                    ns_ = nto_pool.tile([128, 128], FP16, tag="ns")
                    nc.scalar.activation(ns_[:], nto[:], AF.Identity,
                                         accum_out=nf2[:, t:t + 1])

            NCHC = XC // CH              # fb chunks that are pure C-tiles

            def p1_pre(rep):
                """Pure-C fb chunks + their sums matmuls. For rep r+1 this is
                emitted right after rep r's collective so it fills the
                AllReduce window (those buffers' rep-r readers are all done
                at that point)."""
                sums_of_rep[rep] = ps_sums_pool.tile(
                    [C, D], F32, name="sums", tag="sums")
                for k in range(NCHC):
                    p1_dma_chunk(k)
                    for j in range(CH):
                        p1_mm(rep, k * CH + j)

            def ilv_slot(s):
                """Next-rep phase-1 work emitted at P3 slot s of the current
                rep. Every buffer-reusing DMA lands at a slot strictly after
                the current rep's readers of that buffer have been emitted:
                ft chunk fc at slot CH*(fc+1) (S matmuls of chunk fc ran at
                slots <= CH*fc+13); fb A-chunk k at slot CH*(k+1) (A-tile
                TTRs of chunk k ran at slots <= CH*k+13); Gram for tile g at
                slot 2*CH+g, one chunk behind its DMA so the PE never stalls
                on in-flight data."""
                if s % CH == 0:
                    idx = s // CH - 1
                    if 0 <= idx < NFC:
                        ft_dma_chunk(idx)
                    if NCHC <= idx < NCH:
                        p1_dma_chunk(idx)
                g = s - 2 * CH
                if 0 <= g < XC:
                    c_gram(g)

            def p1_drain(rep):
                """Remaining next-rep phase 1 work not covered by the in-loop
                slot schedule (runs during the next collective): leftover ft
                chunks, Gram tiles, the last A chunk and A-tile matmuls."""
                for fc in range((T - 1) // CH, NFC):
                    ft_dma_chunk(fc)
                if NCHC < NCH:
                    p1_dma_chunk(NCH - 1)
                for t in range(XC, T):
                    p1_mm(rep, t)
                    a_norm(t)
                for g in range(max(0, T - 2 * CH), XC):
                    c_gram(g)

            def p1_full(rep):
                # sums matmuls first (they gate the AllReduce), then the
                # Gram norms, which overlap the collective.
                p1_pre(rep)
                for k in range(NCHC, NCH):
                    p1_dma_chunk(k)
                    for j in range(CH):
                        t = k * CH + j
                        p1_mm(rep, t)
                        a_norm(t)
                for fc in range(NFC):
                    ft_dma_chunk(fc)
                for g in range(XC):
                    c_gram(g)

            p1_full(0)

            for rep in range(reps):
                # ---------------- AllReduce (fp16 sums) ----------------
                nc.vector.tensor_copy(sums_h[:], sums_of_rep.pop(rep)[:])
                ar_in = dram_pool.tile([C, D], ARD, tag="ar_in")
                ar_out = dram_pool.tile([C, D], ARD, tag="ar_out")
                nc.scalar.dma_start(ar_in[:], sums_h[:])
                nc.gpsimd.collective_compute(
                    "AllReduce", AL.add,
                    ins=[ar_in.opt()], outs=[ar_out.opt()],
                    replica_groups=[list(range(NCORES))],
                )
                nc.scalar.dma_start(gsum_h[:], ar_out[:])

                # rnf = 1/||f_i|| for THIS rep. Emitted after the AllReduce
                # copy (so the copy doesn't queue behind the norm ops) but
                # before the next rep's norm writers (its Gram TTRs and
                # a_norms are emitted during this rep's phase 3 / drain).
                nc.vector.tensor_scalar_add(nf2[:], nf2[:], 1e-12)
                nc.vector.reciprocal(nf2[:], nf2[:])
                nc.scalar.activation(rnf[:], nf2[:], AF.Sqrt)

                # next rep's phase 1 starts filling the AllReduce window
                ilv = ILV and rep + 1 < reps
                if ilv:
                    p1_pre(rep + 1)

                # ---------------- phase 2: m_hat = gsums/||gsums|| -------
                nc.scalar.activation(gscr[:], gsum_h[:], AF.Square,
                                     accum_out=nm2[:])
                nc.vector.reciprocal(nm2[:], nm2[:])
                nc.scalar.activation(rnm[:], nm2[:], AF.Sqrt)  # 1/||gsum_c||
                nc.vector.tensor_scalar_mul(gmn_f[:], gsum_h[:], rnm[:])
                nc.vector.tensor_copy(gmn_h[:], gmn_f[:])
                if XC:
                    for s in range(4):
                        tp = ps_tp_pool.tile([128, C], F32, tag="tp")
                        nc.tensor.transpose(
                            tp[:], gmn_f[:, s * 128:(s + 1) * 128],
                            id_f[:C, :C])
                        nc.vector.tensor_copy(mnt_v[:, s, :], tp[:])

                # ---------------- phase 3 (+ next rep's phase 1) ---------
                for t in range(T):
                    if t < XC:
                        fc, j = t // FCH, t % FCH
                        wrk = ps_wrk_pool.tile([128, D], F32, name="wrk", tag="wrk")
                        sp = wrk[:, :C]
                        for s in range(4):
                            nc.tensor.matmul(
                                sp,
                                lhsT=ft_bufs[fc][:, s, j * 128:(j + 1) * 128],
                                rhs=mnt_v[:, s, :],
                                start=(s == 0), stop=(s == 3))
                        tto = tto_pool.tile([128, C], FP16, tag="ttoc")
                        eng = nc.gpsimd if POOLTT else nc.vector
                        eng.tensor_tensor(out=tto[:], in0=h_v[:, t, :],
                                          in1=sp, op=AL.mult)
                        if CRED == "dve":
                            nc.vector.tensor_reduce(out=dots[:, t:t + 1],
                                                    in_=tto[:], axis=AX.X,
                                                    op=AL.add)
                        else:
                            cs_ = tto_pool.tile([128, C], FP16, tag="cs")
                            nc.scalar.activation(cs_[:], tto[:], AF.Identity,
                                                 accum_out=dots[:, t:t + 1])
                    else:
                        # capture the fb view BEFORE interleaved next-rep
                        # phase-1 steps rebind the chunk buffer slot
                        fb_t = fb_bufs[t // CH][:, t % CH, :]
                        cent = ps_wrk_pool.tile([128, D], F32, name="wrk", tag="wrk")
                        nc.tensor.matmul(cent[:], lhsT=ht_v[:, t - XC, :],
                                         rhs=gmn_h[:], start=True, stop=True)
                        tto = tto_pool.tile([128, D], FP16, tag="ttoa")
                        nc.vector.tensor_tensor(out=tto[:], in0=fb_t,
                                                in1=cent[:], op=AL.mult)
                        if t % AMOD == 0:
                            nc.vector.tensor_reduce(out=dots[:, t:t + 1],
                                                    in_=tto[:], axis=AX.X,
                                                    op=AL.add)
                        else:
                            ta_ = tto_pool.tile([128, D], FP16, tag="tsa")
                            nc.scalar.activation(ta_[:], tto[:], AF.Identity,
                                                 accum_out=dots[:, t:t + 1])
                    if ilv:
                        ilv_slot(t)
                if ilv:
                    p1_drain(rep + 1)
                elif rep + 1 < reps:
                    p1_full(rep + 1)

                nc.vector.tensor_tensor(out=cos[:], in0=dots[:], in1=rnf[:],
                                        op=AL.mult)
                nc.scalar.dma_start(out_d[:], cos[:])

    nc.compile()
    return nc


def _make_in_maps(feat, label):
    in_maps = []
    idh = np.eye(128, dtype=np.float16)
    for c in range(NCORES):
        sl = slice(c * RPC, (c + 1) * RPC)
        fshard = np.zeros((RPAD, D), dtype=np.float16)
        fshard[:RPC] = feat[sl].astype(np.float16)
        fb = np.ascontiguousarray(
            fshard.reshape(T, 128, D).transpose(1, 0, 2).reshape(128, T * D))
        lab = label[sl]
        h = np.zeros((RPAD, C), dtype=np.float16)
        h[np.arange(RPC), lab] = 1
        h3 = h.reshape(T, 128, C)
        hdev = np.ascontiguousarray(
            h3.transpose(1, 0, 2).reshape(128, T * C))
        m = {"fb": fb, "h": hdev, "idh": idh,
             "idf": np.eye(128, dtype=np.float32)}
        if XC:
            ftr = np.ascontiguousarray(fshard.T)          # [512, RPAD]
            m["ft"] = np.ascontiguousarray(
                ftr.reshape(4, 128, RPAD).transpose(1, 0, 2)
            ).reshape(128, 4 * RPAD)
        if XA:
            m["ht"] = np.ascontiguousarray(
                h3[XC:].transpose(2, 0, 1)).reshape(C, XA * 128)
        in_maps.append(m)
    return in_maps


def kernel(feat: np.ndarray, label: np.ndarray) -> np.ndarray:
    global LAST_RESULTS
    feat = np.ascontiguousarray(np.asarray(feat, dtype=np.float32))
    label = np.asarray(label).astype(np.int64)
    assert feat.shape == (N, D) and label.shape == (N,)

    in_maps = _make_in_maps(feat, label)
    nc = _build_program()
    res = bass_utils.run_bass_kernel_spmd(
        nc, in_maps, core_ids=list(range(NCORES)),
    )
    LAST_RESULTS = res

    out = np.empty(N, dtype=np.float32)
    for c in range(NCORES):
        o = res.results[c]["out"]                   # [128, T]
        out[c * RPC:(c + 1) * RPC] = o.T.ravel()[:RPC]
    return out


if __name__ == "__main__":
    rng = np.random.default_rng(0)
    feat = rng.standard_normal((N, D), dtype=np.float32)
    label = rng.integers(0, C, N)
    cosd = kernel(feat, label)
    sums = np.zeros((C, D), np.float64)
    np.add.at(sums, label, feat.astype(np.float64))
    cnt = np.bincount(label, minlength=C)
    means = sums / np.maximum(cnt, 1)[:, None]
    cent = means[label]
    dot = (feat * cent).sum(1)
    ref = dot / (np.linalg.norm(feat, axis=1) * np.linalg.norm(cent, axis=1))
    err = np.abs(cosd - ref)
    print("max abs err:", err.max(), "max |ref|:", np.abs(ref).max())
    print("scale-rel err:", err.max() / np.abs(ref).max())
